# revision 2
# baseline (speedup 1.0000x reference)
"""Depthwise causal Conv1d (K=16) for x:(4, 2048, 8192) f32 on 8 TRN2 NeuronCores.

Strategy (tensor-parallel over channels, no cross-core communication):
  - Each core owns 256 channels (2048 / 8) for all 4 batches.
  - PO=113 overlap-save windows: the time axis is cut into 128-sample
    windows with stride 113 (15-sample causal halo).  Window rows sit on
    ALL 128 SBUF partitions (in natural time order, no reversal):
        X[p, c, (b, j)] = xpad[b, c, 113*j + p],  xpad = [15 zeros] ++ x
        y[b, c, 113*j + m] = sum_p A[p, m] * X[p, (b, j)]
        A[p, m] = w[p - m]  for 0 <= p - m <= 15   (banded, 128 x 113)
  - The band is evaluated as TWO concurrent matmuls per channel in
    disjoint PE-array column strips (no PSUM accumulation, no races):
        left : lhsT aL[0:79, 0:64]    rhs X[0:79]    -> psum[0:64]
        right: lhsT aR[64:128, 0:49]  rhs X[64:128]  -> psum[64:113]
    (for m < 64 the band p in [m, m+15] lies in [0, 79); for m >= 64 it
    lies in [64, 128) -- exact split, PO=113 is maximal for PIN=128.)
  - Every DMA covers 128 (loads) or 113 (stores) partitions with
    multi-KB contiguous per-partition runs and >= 1 MiB per transfer,
    spread round-robin over the three DMA queues (gpsimd/sync/scalar).
  - Everything is bf16 on the wire; PSUM accumulates in f32; the
    PSUM->SBUF drain downcasts to bf16 (alternating vector/scalar).
  - Bias is added on the host (it is identically zero in this problem).

The host does the sharding + window-layout transposes with numpy; the
device kernel sees only dense p-major arrays.
"""

import sys

import ml_dtypes
import numpy as np
from numpy.lib.stride_tricks import sliding_window_view

if "/opt/trn_rl_repo" not in sys.path:
    sys.path.insert(0, "/opt/trn_rl_repo")

import concourse.bacc as bacc
import concourse.mybir as mybir
import concourse.tile as tile
from concourse.bass_utils import run_bass_kernel_spmd

F32 = mybir.dt.float32
BF16 = mybir.dt.bfloat16
NP_BF16 = np.dtype(ml_dtypes.bfloat16)
ACT_COPY = mybir.ActivationFunctionType.Copy

N_CORES = 8
B = 4              # batch
DIM = 2048         # channels
T = 8192           # time
K = 16             # conv taps
C = DIM // N_CORES # channels per core = 256
PO = 113           # outputs per window
PIN = 128          # window rows = PO + K - 1
NJ = -(-T // PO)   # windows per (batch, channel) = 73 (73*113 = 8249 >= 8192)
Q = B * NJ         # columns per channel = 292
ML = 64            # left matmul output cols (m in [0, 64))
PL = ML + K - 1    # left matmul contraction rows = 79
MR = PO - ML       # right matmul output cols = 49 (m in [64, 113))
CH = 32            # channels per chunk
NCHUNK = C // CH   # 8
ALPAD = 16         # a_l DRAM free-dim pad: breaks 32768B pow2 partition stride


_compiled_nc = None


def _build_kernel():
    nc = bacc.Bacc(None)

    xin = nc.declare_dram_parameter("xin", [PIN, C, Q], BF16, isOutput=False)
    a_l = nc.declare_dram_parameter("a_l", [PL, C * ML + ALPAD], BF16, isOutput=False)
    a_r = nc.declare_dram_parameter("a_r", [PIN - ML, C * MR], BF16, isOutput=False)
    yout = nc.declare_dram_parameter("yout", [PO, C, Q], BF16, isOutput=True)

    dma_engs = [None, None, None]

    with tile.TileContext(nc) as tc:
        dma_engs = [nc.gpsimd, nc.sync, nc.scalar]
        with (
            tc.tile_pool(name="apool", bufs=1) as apool,
            tc.tile_pool(name="xpool", bufs=3) as xpool,
            tc.tile_pool(name="opool", bufs=3) as opool,
            tc.tile_pool(name="psum", bufs=8, space="PSUM") as pspool,
        ):
            a_l_t = apool.tile([PL, C * ML], BF16)
            a_r_t = apool.tile([PIN, C * MR], BF16)
            nc.sync.dma_start(out=a_l_t[:, :], in_=a_l[:, 0 : C * ML])
            nc.scalar.dma_start(out=a_r_t[ML:PIN, :], in_=a_r[:, :])

            for chunk in range(NCHUNK):
                c0 = chunk * CH
                x_t = xpool.tile([PIN, CH * Q], BF16)
                o_t = opool.tile([PIN, CH * Q], BF16)

                dma_engs[chunk % 3].dma_start(
                    out=x_t[:].rearrange("p (c q) -> p c q", c=CH),
                    in_=xin[:, c0 : c0 + CH, :],
                )

                for i in range(CH):
                    c = c0 + i
                    ps = pspool.tile([PIN, Q], F32)
                    nc.tensor.matmul(
                        ps[0:ML, :],
                        a_l_t[0:PL, c * ML : (c + 1) * ML],
                        x_t[0:PL, i * Q : (i + 1) * Q],
                        start=True,
                        stop=True,
                    )
                    nc.tensor.matmul(
                        ps[ML:PO, :],
                        a_r_t[ML:PIN, c * MR : (c + 1) * MR],
                        x_t[ML:PIN, i * Q : (i + 1) * Q],
                        start=True,
                        stop=True,
                    )
                    dst = o_t[0:PO, i * Q : (i + 1) * Q]
                    if i % 2 == 0:
                        nc.vector.tensor_copy(dst, ps[0:PO, :])
                    else:
                        nc.scalar.activation(dst, ps[0:PO, :], ACT_COPY)

                dma_engs[(chunk + 1) % 3].dma_start(
                    out=yout[:, c0 : c0 + CH, :],
                    in_=o_t[0:PO, :].rearrange("p (c q) -> p c q", c=CH),
                )

    nc.compile()
    return nc


def _get_nc():
    global _compiled_nc
    if _compiled_nc is None:
        _compiled_nc = _build_kernel()
    return _compiled_nc


def _prep_core(x, weight, core):
    """Build the per-core input map (numpy only)."""
    cs = slice(core * C, (core + 1) * C)
    xs = x[:, cs, :]                       # [B, C, T]
    w = weight[cs, 0, :]                   # [C, K]

    # X[p, c, (b, j)] = xpad[b, c, 113*j + p]; xpad = [15 zeros] ++ x ++ zeros
    xpad = np.zeros((B, C, K - 1 + PO * (NJ - 1) + PIN), dtype=np.float32)
    xpad[:, :, K - 1 : K - 1 + T] = xs
    sw = sliding_window_view(xpad, PIN, axis=2)[:, :, ::PO, :]  # [B,C,NJ,128]
    xin = np.ascontiguousarray(
        sw.transpose(3, 1, 0, 2).astype(NP_BF16).reshape(PIN, C, Q)
    )

    # A[p, m] = w[p - m] for 0 <= p - m <= 15
    pl = np.arange(PL)[:, None]
    mlg = np.arange(ML)[None, :]
    bandl = (pl - mlg >= 0) & (pl - mlg <= K - 1)
    al = np.where(bandl[None], w[:, np.clip(pl - mlg, 0, K - 1)], 0.0)  # [C,79,64]
    a_l = np.zeros((PL, C * ML + ALPAD), dtype=NP_BF16)
    a_l[:, 0 : C * ML] = al.transpose(1, 0, 2).astype(NP_BF16).reshape(PL, C * ML)

    pr = np.arange(PIN - ML)[:, None]
    mr = np.arange(MR)[None, :]
    bandr = (pr - mr >= 0) & (pr - mr <= K - 1)
    ar = np.where(bandr[None], w[:, np.clip(pr - mr, 0, K - 1)], 0.0)  # [C,64,49]
    a_r = np.ascontiguousarray(
        ar.transpose(1, 0, 2).astype(NP_BF16).reshape(PIN - ML, C * MR)
    )

    return {"xin": xin, "a_l": a_l, "a_r": a_r}


def run(x, weight, bias, trace=False):
    nc = _get_nc()
    in_maps = [_prep_core(x, weight, core) for core in range(N_CORES)]
    res = run_bass_kernel_spmd(nc, in_maps, list(range(N_CORES)), trace=trace)

    y = np.empty((B, DIM, T), dtype=np.float32)
    for core in range(N_CORES):
        yp = np.asarray(res.results[core]["yout"]).astype(np.float32)  # [PO,C,Q]
        # yp[m, c, b*NJ + j] -> y[b, c, 113*j + m]
        yc = yp.reshape(PO, C, B, NJ).transpose(2, 1, 3, 0).reshape(B, C, NJ * PO)
        y[:, core * C : (core + 1) * C, :] = yc[:, :, :T]
    if np.any(bias):
        y += bias[None, :, None]
    return y, res


def kernel(x, weight, bias):
    y, _ = run(
        np.asarray(x, dtype=np.float32),
        np.asarray(weight, dtype=np.float32),
        np.asarray(bias, dtype=np.float32),
    )
    return y


# revision 7
# speedup vs baseline: 1.0145x; 1.0145x over previous
"""Depthwise causal Conv1d (K=16) for x:(4, 2048, 8192) f32 on 8 TRN2 NeuronCores.

Strategy (tensor-parallel over channels, no cross-core communication):
  - Each core owns 256 channels (2048 / 8) for all 4 batches.
  - PO=113 overlap-save windows: the time axis is cut into 128-sample
    windows with stride 113 (15-sample causal halo).  Window rows sit on
    ALL 128 SBUF partitions (in natural time order, no reversal):
        X[p, c, (b, j)] = xpad[b, c, 113*j + p],  xpad = [15 zeros] ++ x
        y[b, c, 113*j + m] = sum_p A[p, m] * X[p, (b, j)]
        A[p, m] = w[p - m]  for 0 <= p - m <= 15   (banded, 128 x 113)
  - The band is evaluated as TWO concurrent matmuls per channel in
    disjoint PE-array column strips (no PSUM accumulation, no races):
        left : lhsT aL[0:79, 0:64]    rhs X[0:79]    -> psum[0:64]
        right: lhsT aR[64:128, 0:49]  rhs X[64:128]  -> psum[64:113]
    (for m < 64 the band p in [m, m+15] lies in [0, 79); for m >= 64 it
    lies in [64, 128) -- exact split, PO=113 is maximal for PIN=128.)
  - Every DMA covers 128 (loads) or 113 (stores) partitions with
    multi-KB contiguous per-partition runs and >= 1 MiB per transfer,
    spread round-robin over the three DMA queues (gpsimd/sync/scalar).
  - Everything is bf16 on the wire; PSUM accumulates in f32; the
    PSUM->SBUF drain downcasts to bf16 (alternating vector/scalar).
  - Bias is added on the host (it is identically zero in this problem).

The host does the sharding + window-layout transposes with numpy; the
device kernel sees only dense p-major arrays.
"""

import sys

import ml_dtypes
import numpy as np
from numpy.lib.stride_tricks import sliding_window_view

if "/opt/trn_rl_repo" not in sys.path:
    sys.path.insert(0, "/opt/trn_rl_repo")

import concourse.bacc as bacc
import concourse.mybir as mybir
import concourse.tile as tile
from concourse.bass_utils import run_bass_kernel_spmd

F32 = mybir.dt.float32
BF16 = mybir.dt.bfloat16
NP_BF16 = np.dtype(ml_dtypes.bfloat16)
ACT_COPY = mybir.ActivationFunctionType.Copy

N_CORES = 8
B = 4              # batch
DIM = 2048         # channels
T = 8192           # time
K = 16             # conv taps
C = DIM // N_CORES # channels per core = 256
PO = 113           # outputs per window
PIN = 128          # window rows = PO + K - 1
NJ = -(-T // PO)   # windows per (batch, channel) = 73 (73*113 = 8249 >= 8192)
Q = B * NJ         # columns per channel = 292
ML = 64            # left matmul output cols (m in [0, 64))
PL = ML + K - 1    # left matmul contraction rows = 79
MR = PO - ML       # right matmul output cols = 49 (m in [64, 113))
CH = 32            # channels per chunk
NCHUNK = C // CH   # 8
ALPAD = 16         # a_l DRAM free-dim pad: breaks 32768B pow2 partition stride


_compiled_nc = None


def _build_kernel():
    nc = bacc.Bacc(None)

    # All DMAs are 2-D APs (partition x one contiguous run): a 3-D AP makes
    # the DGE iterate per-(p,c) entries at ~1/us and transfers trickle out.
    xin = nc.declare_dram_parameter("xin", [PIN, C * Q], BF16, isOutput=False)
    a_l = nc.declare_dram_parameter("a_l", [PL, C * ML + ALPAD], BF16, isOutput=False)
    a_r = nc.declare_dram_parameter("a_r", [PIN - ML, C * MR], BF16, isOutput=False)
    yout = nc.declare_dram_parameter("yout", [PO, C * Q], BF16, isOutput=True)

    dma_engs = [None, None, None]

    with tile.TileContext(nc) as tc:
        dma_engs = [nc.gpsimd, nc.sync, nc.scalar]
        with (
            tc.tile_pool(name="apool", bufs=1) as apool,
            tc.tile_pool(name="xpool", bufs=3) as xpool,
            tc.tile_pool(name="opool", bufs=3) as opool,
            tc.tile_pool(name="psum", bufs=8, space="PSUM") as pspool,
        ):
            a_l_t = apool.tile([PL, C * ML], BF16)
            a_r_t = apool.tile([PIN, C * MR], BF16)
            nc.sync.dma_start(out=a_l_t[:, :], in_=a_l[:, 0 : C * ML])
            nc.scalar.dma_start(out=a_r_t[ML:PIN, :], in_=a_r[:, :])

            for chunk in range(NCHUNK):
                c0 = chunk * CH
                x_t = xpool.tile([PIN, CH * Q], BF16)
                o_t = opool.tile([PIN, CH * Q], BF16)

                dma_engs[chunk % 3].dma_start(
                    out=x_t[:, :],
                    in_=xin[:, c0 * Q : (c0 + CH) * Q],
                )

                for i in range(CH):
                    c = c0 + i
                    ps = pspool.tile([PIN, Q], F32)
                    nc.tensor.matmul(
                        ps[0:ML, :],
                        a_l_t[0:PL, c * ML : (c + 1) * ML],
                        x_t[0:PL, i * Q : (i + 1) * Q],
                        start=True,
                        stop=True,
                    )
                    nc.tensor.matmul(
                        ps[ML:PO, :],
                        a_r_t[ML:PIN, c * MR : (c + 1) * MR],
                        x_t[ML:PIN, i * Q : (i + 1) * Q],
                        start=True,
                        stop=True,
                    )
                    dst = o_t[0:PO, i * Q : (i + 1) * Q]
                    if i % 2 == 0:
                        nc.vector.tensor_copy(dst, ps[0:PO, :])
                    else:
                        nc.scalar.activation(dst, ps[0:PO, :], ACT_COPY)

                dma_engs[(chunk + 1) % 3].dma_start(
                    out=yout[:, c0 * Q : (c0 + CH) * Q],
                    in_=o_t[0:PO, :],
                )

    nc.compile()
    return nc


def _get_nc():
    global _compiled_nc
    if _compiled_nc is None:
        _compiled_nc = _build_kernel()
    return _compiled_nc


def _prep_core(x, weight, core):
    """Build the per-core input map (numpy only)."""
    cs = slice(core * C, (core + 1) * C)
    xs = x[:, cs, :]                       # [B, C, T]
    w = weight[cs, 0, :]                   # [C, K]

    # X[p, c, (b, j)] = xpad[b, c, 113*j + p]; xpad = [15 zeros] ++ x ++ zeros
    xpad = np.zeros((B, C, K - 1 + PO * (NJ - 1) + PIN), dtype=np.float32)
    xpad[:, :, K - 1 : K - 1 + T] = xs
    sw = sliding_window_view(xpad, PIN, axis=2)[:, :, ::PO, :]  # [B,C,NJ,128]
    xin = np.ascontiguousarray(
        sw.transpose(3, 1, 0, 2).astype(NP_BF16).reshape(PIN, C * Q)
    )

    # A[p, m] = w[p - m] for 0 <= p - m <= 15
    pl = np.arange(PL)[:, None]
    mlg = np.arange(ML)[None, :]
    bandl = (pl - mlg >= 0) & (pl - mlg <= K - 1)
    al = np.where(bandl[None], w[:, np.clip(pl - mlg, 0, K - 1)], 0.0)  # [C,79,64]
    a_l = np.zeros((PL, C * ML + ALPAD), dtype=NP_BF16)
    a_l[:, 0 : C * ML] = al.transpose(1, 0, 2).astype(NP_BF16).reshape(PL, C * ML)

    pr = np.arange(PIN - ML)[:, None]
    mr = np.arange(MR)[None, :]
    bandr = (pr - mr >= 0) & (pr - mr <= K - 1)
    ar = np.where(bandr[None], w[:, np.clip(pr - mr, 0, K - 1)], 0.0)  # [C,64,49]
    a_r = np.ascontiguousarray(
        ar.transpose(1, 0, 2).astype(NP_BF16).reshape(PIN - ML, C * MR)
    )

    return {"xin": xin, "a_l": a_l, "a_r": a_r}


def run(x, weight, bias, trace=False):
    nc = _get_nc()
    in_maps = [_prep_core(x, weight, core) for core in range(N_CORES)]
    res = run_bass_kernel_spmd(nc, in_maps, list(range(N_CORES)), trace=trace)

    y = np.empty((B, DIM, T), dtype=np.float32)
    for core in range(N_CORES):
        yp = np.asarray(res.results[core]["yout"]).astype(np.float32)  # [PO,C*Q]
        # yp[m, c, b*NJ + j] -> y[b, c, 113*j + m]
        yc = yp.reshape(PO, C, B, NJ).transpose(2, 1, 3, 0).reshape(B, C, NJ * PO)
        y[:, core * C : (core + 1) * C, :] = yc[:, :, :T]
    if np.any(bias):
        y += bias[None, :, None]
    return y, res


def kernel(x, weight, bias):
    y, _ = run(
        np.asarray(x, dtype=np.float32),
        np.asarray(weight, dtype=np.float32),
        np.asarray(bias, dtype=np.float32),
    )
    return y


# revision 8
# speedup vs baseline: 2.9250x; 2.8832x over previous
"""Depthwise causal Conv1d (K=16) for x:(4, 2048, 8192) f32 on 8 TRN2 NeuronCores.

Strategy (tensor-parallel over channels, no cross-core communication):
  - Each core owns 256 channels (2048 / 8) for all 4 batches.
  - PO=113 overlap-save windows: the time axis is cut into 128-sample
    windows with stride 113 (15-sample causal halo).  Window rows sit on
    ALL 128 SBUF partitions (natural time order, no reversal):
        X[p, c, (b, j)] = xpad[b, c, 113*j + p],  xpad = [15 zeros] ++ x
        y[b, c, 113*j + m] = sum_p A[p, m] * X[p, (b, j)]
        A[p, m] = w[p - m]  for 0 <= p - m <= 15   (banded, 128 x 113)
  - The band is evaluated as TWO concurrent matmuls per channel in
    disjoint PE-array strips (no PSUM accumulation, no write races):
        left : lhsT A[0:79, 0:64]     rhs X[0:79]    -> psum[0:64]
        right: lhsT A[64:128, 64:128] rhs X[64:128]  -> psum[64:128]
    (for m < 64 the band p in [m, m+15] lies in [0, 79); for m >= 64 it
    lies in [64, 128).  The right block's columns m in [113, 128) hold
    zero weights so psum rows 113..127 are written zeros -- that keeps
    every PSUM drain and store DMA at full 128 partitions.)
  - EVERY DMA covers exactly 128 partitions with one contiguous
    multi-KB run per partition (2-D access patterns): partial-partition
    or multi-dim APs collapse onto a single SDMA engine (~27 GB/s)
    instead of spreading across all 16 (~420 GB/s).  aL and aR are
    zero-padded into one [128, C*128] blob for this reason.
  - Everything is bf16 on the wire; PSUM accumulates in f32; the
    PSUM->SBUF drain downcasts to bf16 (alternating vector/scalar).
  - Bias is added on the host (it is identically zero in this problem).

The host does the sharding + window-layout transposes with numpy; the
device kernel sees only dense p-major arrays.
"""

import sys

import ml_dtypes
import numpy as np
from numpy.lib.stride_tricks import sliding_window_view

if "/opt/trn_rl_repo" not in sys.path:
    sys.path.insert(0, "/opt/trn_rl_repo")

import concourse.bacc as bacc
import concourse.mybir as mybir
import concourse.tile as tile
from concourse.bass_utils import run_bass_kernel_spmd

F32 = mybir.dt.float32
BF16 = mybir.dt.bfloat16
NP_BF16 = np.dtype(ml_dtypes.bfloat16)
ACT_COPY = mybir.ActivationFunctionType.Copy

N_CORES = 8
B = 4              # batch
DIM = 2048         # channels
T = 8192           # time
K = 16             # conv taps
C = DIM // N_CORES # channels per core = 256
PO = 113           # outputs per window
PIN = 128          # window rows = PO + K - 1
NJ = -(-T // PO)   # windows per (batch, channel) = 73 (73*113 = 8249 >= 8192)
Q = B * NJ         # columns per channel = 292
ML = 64            # left matmul output cols (m in [0, 64))
PL = ML + K - 1    # left matmul contraction rows = 79
MR = PO - ML       # real right outputs = 49 (cols 49..63 of aR are zero)
AW = 2 * ML        # A blob cols per channel = 128 (64 aL + 64 aR)
CH = 32            # channels per chunk
NCHUNK = C // CH   # 8
ABPAD = 16         # A blob free-dim pad: breaks 65536B pow2 partition stride


_compiled_nc = None


def _build_kernel():
    nc = bacc.Bacc(None)

    xin = nc.declare_dram_parameter("xin", [PIN, C * Q], BF16, isOutput=False)
    ab = nc.declare_dram_parameter("ab", [PIN, C * AW + ABPAD], BF16, isOutput=False)
    yout = nc.declare_dram_parameter("yout", [PIN, C * Q], BF16, isOutput=True)

    with tile.TileContext(nc) as tc:
        engs = [nc.gpsimd, nc.sync, nc.scalar]
        with (
            tc.tile_pool(name="apool", bufs=3) as apool,
            tc.tile_pool(name="xpool", bufs=3) as xpool,
            tc.tile_pool(name="opool", bufs=3) as opool,
            tc.tile_pool(name="psum", bufs=8, space="PSUM") as pspool,
        ):
            for chunk in range(NCHUNK):
                c0 = chunk * CH
                x_t = xpool.tile([PIN, CH * Q], BF16)
                ab_t = apool.tile([PIN, CH * AW], BF16)
                o_t = opool.tile([PIN, CH * Q], BF16)

                engs[chunk % 3].dma_start(
                    out=x_t[:, :], in_=xin[:, c0 * Q : (c0 + CH) * Q]
                )
                engs[(chunk + 1) % 3].dma_start(
                    out=ab_t[:, :], in_=ab[:, c0 * AW : (c0 + CH) * AW]
                )

                for i in range(CH):
                    ps = pspool.tile([PIN, Q], F32)
                    nc.tensor.matmul(
                        ps[0:ML, :],
                        ab_t[0:PL, i * AW : i * AW + ML],
                        x_t[0:PL, i * Q : (i + 1) * Q],
                        start=True,
                        stop=True,
                    )
                    nc.tensor.matmul(
                        ps[ML:PIN, :],
                        ab_t[ML:PIN, i * AW + ML : (i + 1) * AW],
                        x_t[ML:PIN, i * Q : (i + 1) * Q],
                        start=True,
                        stop=True,
                    )
                    dst = o_t[:, i * Q : (i + 1) * Q]
                    if i % 2 == 0:
                        nc.vector.tensor_copy(dst, ps[:, :])
                    else:
                        nc.scalar.activation(dst, ps[:, :], ACT_COPY)

                engs[(chunk + 2) % 3].dma_start(
                    out=yout[:, c0 * Q : (c0 + CH) * Q], in_=o_t[:, :]
                )

    nc.compile()
    return nc


def _get_nc():
    global _compiled_nc
    if _compiled_nc is None:
        _compiled_nc = _build_kernel()
    return _compiled_nc


def _prep_core(x, weight, core):
    """Build the per-core input map (numpy only)."""
    cs = slice(core * C, (core + 1) * C)
    xs = x[:, cs, :]                       # [B, C, T]
    w = weight[cs, 0, :]                   # [C, K]

    # X[p, c, (b, j)] = xpad[b, c, 113*j + p]; xpad = [15 zeros] ++ x ++ zeros
    xpad = np.zeros((B, C, K - 1 + PO * (NJ - 1) + PIN), dtype=np.float32)
    xpad[:, :, K - 1 : K - 1 + T] = xs
    sw = sliding_window_view(xpad, PIN, axis=2)[:, :, ::PO, :]  # [B,C,NJ,128]
    xin = np.ascontiguousarray(
        sw.transpose(3, 1, 0, 2).astype(NP_BF16).reshape(PIN, C * Q)
    )

    # A[p, m] = w[p - m] for 0 <= p - m <= 15, packed as [128, c, 64+64]:
    #   cols [0:64)  rows [0:79)   = aL (m in [0, 64))
    #   cols [64:128) rows [64:128) = aR (m in [64, 113); m in [113,128) zero)
    pl = np.arange(PL)[:, None]
    mlg = np.arange(ML)[None, :]
    bandl = (pl - mlg >= 0) & (pl - mlg <= K - 1)
    al = np.where(bandl[None], w[:, np.clip(pl - mlg, 0, K - 1)], 0.0)  # [C,79,64]

    pr = np.arange(PIN - ML)[:, None]
    mr = np.arange(MR)[None, :]
    bandr = (pr - mr >= 0) & (pr - mr <= K - 1)
    ar = np.where(bandr[None], w[:, np.clip(pr - mr, 0, K - 1)], 0.0)  # [C,64,49]

    abf = np.zeros((PIN, C, AW), dtype=NP_BF16)
    abf[0:PL, :, 0:ML] = al.transpose(1, 0, 2).astype(NP_BF16)
    abf[ML:PIN, :, ML : ML + MR] = ar.transpose(1, 0, 2).astype(NP_BF16)
    ab = np.zeros((PIN, C * AW + ABPAD), dtype=NP_BF16)
    ab[:, 0 : C * AW] = abf.reshape(PIN, C * AW)

    return {"xin": xin, "ab": ab}


def run(x, weight, bias, trace=False):
    nc = _get_nc()
    in_maps = [_prep_core(x, weight, core) for core in range(N_CORES)]
    res = run_bass_kernel_spmd(nc, in_maps, list(range(N_CORES)), trace=trace)

    y = np.empty((B, DIM, T), dtype=np.float32)
    for core in range(N_CORES):
        yp = np.asarray(res.results[core]["yout"]).astype(np.float32)  # [128,C*Q]
        # yp[m, c, b*NJ + j] -> y[b, c, 113*j + m]  (rows 113..127 are pad)
        yc = (
            yp.reshape(PIN, C, B, NJ)[0:PO]
            .transpose(2, 1, 3, 0)
            .reshape(B, C, NJ * PO)
        )
        y[:, core * C : (core + 1) * C, :] = yc[:, :, :T]
    if np.any(bias):
        y += bias[None, :, None]
    return y, res


def kernel(x, weight, bias):
    y, _ = run(
        np.asarray(x, dtype=np.float32),
        np.asarray(weight, dtype=np.float32),
        np.asarray(bias, dtype=np.float32),
    )
    return y


# revision 12
# speedup vs baseline: 5.0594x; 1.7297x over previous
"""Depthwise causal Conv1d (K=16) for x:(4, 2048, 8192) f32 on 8 TRN2 NeuronCores.

Strategy (tensor-parallel over channels, no cross-core communication):
  - Each core owns 256 channels (2048 / 8) for all 4 batches.
  - PO=113 overlap-save windows: the time axis is cut into 128-sample
    windows with stride 113 (15-sample causal halo).  Window rows sit on
    ALL 128 SBUF partitions (natural time order, no reversal):
        X[p, c, (b, j)] = xpad[b, c, 113*j + p],  xpad = [15 zeros] ++ x
        y[b, c, 113*j + m] = sum_p A[p, m] * X[p, (b, j)]
        A[p, m] = w[p - m]  for 0 <= p - m <= 15   (banded, 128 x 113)
  - The band is evaluated as ONE [128, 128] stationary matmul per
    channel (columns m in [113, 128) hold zero weights, so psum rows
    113..127 are written zeros -- that keeps every PSUM drain and store
    DMA at full 128 partitions).  Splitting the band into two smaller
    matmuls is NOT faster: any tile whose rounded row-size is 128 blocks
    LDWEIGHTS pull-ahead, so the pieces serialize anyway and just double
    the instruction count (measured 259us of PE time vs ~90us fused).
  - EVERY DMA covers exactly 128 partitions with one contiguous
    multi-KB run per partition (2-D access patterns): partial-partition
    or multi-dim APs collapse onto a single SDMA engine (~27 GB/s)
    instead of spreading across all 16 (~420 GB/s).  aL and aR are
    zero-padded into one [128, C*128] blob for this reason.
  - Everything is bf16 on the wire; PSUM accumulates in f32; the
    PSUM->SBUF drain downcasts to bf16 (alternating vector/scalar).
  - Bias is added on the host (it is identically zero in this problem).

The host does the sharding + window-layout transposes with numpy; the
device kernel sees only dense p-major arrays.
"""

import sys

import ml_dtypes
import numpy as np
from numpy.lib.stride_tricks import sliding_window_view

if "/opt/trn_rl_repo" not in sys.path:
    sys.path.insert(0, "/opt/trn_rl_repo")

import concourse.bacc as bacc
import concourse.mybir as mybir
import concourse.tile as tile
from concourse.bass_utils import run_bass_kernel_spmd

F32 = mybir.dt.float32
BF16 = mybir.dt.bfloat16
NP_BF16 = np.dtype(ml_dtypes.bfloat16)
ACT_COPY = mybir.ActivationFunctionType.Copy

N_CORES = 8
B = 4              # batch
DIM = 2048         # channels
T = 8192           # time
K = 16             # conv taps
C = DIM // N_CORES # channels per core = 256
PO = 113           # outputs per window
PIN = 128          # window rows = PO + K - 1
NJ = -(-T // PO)   # windows per (batch, channel) = 73 (73*113 = 8249 >= 8192)
Q = B * NJ         # columns per channel = 292
AW = PIN           # A blob cols per channel = 128 (cols 113..127 zero)
CH = 32            # channels per chunk
NCHUNK = C // CH   # 8
ABPAD = 16         # A blob free-dim pad: breaks 65536B pow2 partition stride


_compiled_nc = None


def _build_kernel():
    nc = bacc.Bacc(None)

    xin = nc.declare_dram_parameter("xin", [PIN, C * Q], BF16, isOutput=False)
    ab = nc.declare_dram_parameter("ab", [PIN, C * AW + ABPAD], BF16, isOutput=False)
    yout = nc.declare_dram_parameter("yout", [PIN, C * Q], BF16, isOutput=True)

    with tile.TileContext(nc) as tc:
        engs = [nc.gpsimd, nc.sync, nc.scalar]
        with (
            tc.tile_pool(name="apool", bufs=3) as apool,
            tc.tile_pool(name="xpool", bufs=3) as xpool,
            tc.tile_pool(name="opool", bufs=3) as opool,
            tc.tile_pool(name="psum", bufs=8, space="PSUM") as pspool,
        ):
            for chunk in range(NCHUNK):
                c0 = chunk * CH
                x_t = xpool.tile([PIN, CH * Q], BF16)
                ab_t = apool.tile([PIN, CH * AW], BF16)
                o_t = opool.tile([PIN, CH * Q], BF16)

                engs[chunk % 3].dma_start(
                    out=x_t[:, :], in_=xin[:, c0 * Q : (c0 + CH) * Q]
                )
                engs[(chunk + 1) % 3].dma_start(
                    out=ab_t[:, :], in_=ab[:, c0 * AW : (c0 + CH) * AW]
                )

                for i in range(CH):
                    ps = pspool.tile([PIN, Q], F32)
                    nc.tensor.matmul(
                        ps[:, :],
                        ab_t[:, i * AW : (i + 1) * AW],
                        x_t[:, i * Q : (i + 1) * Q],
                        start=True,
                        stop=True,
                    )
                    dst = o_t[:, i * Q : (i + 1) * Q]
                    if i % 2 == 0:
                        nc.vector.tensor_copy(dst, ps[:, :])
                    else:
                        nc.scalar.activation(dst, ps[:, :], ACT_COPY)

                engs[(chunk + 2) % 3].dma_start(
                    out=yout[:, c0 * Q : (c0 + CH) * Q], in_=o_t[:, :]
                )

    nc.compile()
    return nc


def _get_nc():
    global _compiled_nc
    if _compiled_nc is None:
        _compiled_nc = _build_kernel()
    return _compiled_nc


def _prep_core(x, weight, core):
    """Build the per-core input map (numpy only)."""
    cs = slice(core * C, (core + 1) * C)
    xs = x[:, cs, :]                       # [B, C, T]
    w = weight[cs, 0, :]                   # [C, K]

    # X[p, c, (b, j)] = xpad[b, c, 113*j + p]; xpad = [15 zeros] ++ x ++ zeros
    xpad = np.zeros((B, C, K - 1 + PO * (NJ - 1) + PIN), dtype=np.float32)
    xpad[:, :, K - 1 : K - 1 + T] = xs
    sw = sliding_window_view(xpad, PIN, axis=2)[:, :, ::PO, :]  # [B,C,NJ,128]
    xin = np.ascontiguousarray(
        sw.transpose(3, 1, 0, 2).astype(NP_BF16).reshape(PIN, C * Q)
    )

    # A[p, m] = w[p - m] for 0 <= p - m <= 15 and m < PO, else 0
    pi = np.arange(PIN)[:, None]
    mi = np.arange(AW)[None, :]
    band = (pi - mi >= 0) & (pi - mi <= K - 1) & (mi < PO)
    av = np.where(band[None], w[:, np.clip(pi - mi, 0, K - 1)], 0.0)  # [C,128,128]
    ab = np.zeros((PIN, C * AW + ABPAD), dtype=NP_BF16)
    ab[:, 0 : C * AW] = (
        av.transpose(1, 0, 2).astype(NP_BF16).reshape(PIN, C * AW)
    )

    return {"xin": xin, "ab": ab}


def run(x, weight, bias, trace=False):
    nc = _get_nc()
    in_maps = [_prep_core(x, weight, core) for core in range(N_CORES)]
    res = run_bass_kernel_spmd(nc, in_maps, list(range(N_CORES)), trace=trace)

    y = np.empty((B, DIM, T), dtype=np.float32)
    for core in range(N_CORES):
        yp = np.asarray(res.results[core]["yout"]).astype(np.float32)  # [128,C*Q]
        # yp[m, c, b*NJ + j] -> y[b, c, 113*j + m]  (rows 113..127 are pad)
        yc = (
            yp.reshape(PIN, C, B, NJ)[0:PO]
            .transpose(2, 1, 3, 0)
            .reshape(B, C, NJ * PO)
        )
        y[:, core * C : (core + 1) * C, :] = yc[:, :, :T]
    if np.any(bias):
        y += bias[None, :, None]
    return y, res


def kernel(x, weight, bias):
    y, _ = run(
        np.asarray(x, dtype=np.float32),
        np.asarray(weight, dtype=np.float32),
        np.asarray(bias, dtype=np.float32),
    )
    return y
